# revision 22
# baseline (speedup 1.0000x reference)
"""EncDec ConvLSTM kernel for 8 Trainium2 NeuronCores.

Sharding: 8 cores = 4 (batch) x 2 (spatial row-halves). Each core owns 32
output rows; a 3-row halo is refreshed by a pairwise AllReduce exchange
every 3rd step, so per-step redundant compute shrinks from the old
no-comms scheme's 880 rows/core to 661. Row-half 1 cores receive a
vertically flipped image and ky-flipped conv weights, so a single SPMD
program serves all cores. Ghost rows are recovered rank-agnostically as
(pair-sum - mine); the exchange is issued right after the boundary tile
(computed first on exchange steps) so its ~13us latency hides under ~2.5
steps of compute.

Conv3x3 maps to PE matmuls over pixels (N up to 512, bf16), all with
full K=128 stationary loads so LDWEIGHTS hides behind in-flight matmuls.
Two double-buffered state tiles per step:
  R  = [h (parts 0:64) | h col-shifted +2 (parts 64:128)]
  R2 = [h (parts 0:64) | h row-shifted +1 (parts 64:128)]
Per row-tile and M-tile, 6 matmuls: x-im2col (K=72 zero-padded to 128),
3 paired-kx taps on R at row offsets 0/1/2, middle-column ky=0/1 as one
K=128 MM on R2, and ky=2 zero-padded on R. The three shift copies per
tile are flat contiguous-span DMAs (pad columns are zero, so a uniform
address delta realizes the shift), which issue far cheaper than strided
row views. The per-step x im2col loads are 3 batched DMAs (one per ky)
using a custom overlapping-window access pattern.

Epilogue runs entirely in tanh form (sigmoid(z) = 0.5*tanh(z/2)+0.5 with
the /2 folded into ACT scale/bias and the +1/x0.5 fixups into
scalar_tensor_tensor ops; stored h and c carry a 2x factor compensated
by pre-halved h-tap weights), 3 ACT ops per tile. The (f+1)*c product
runs on gpsimd to balance DVE load. Each tile's tail (tanh(c), h write,
shifts) is emitted one tile later so the ACT FIFO never head-of-line
blocks on the DVE c-update chain; decoder out-convs are interleaved
between gate tiles and their outputs batched to one DMA per step. A
32-matmul warm-up raises the HAM clock gate before the real work.
"""

import os
import sys

import numpy as np

for _p in ("/opt/trn_rl_repo", "/root/.axon_site/_ro/trn_rl_repo"):
    if os.path.isdir(_p) and _p not in sys.path:
        sys.path.append(_p)

T = 10
F = 8
HD = 64
HS = 64
WS = 64
NCORES = 8
PW = 66   # padded grid width
# LEAD is odd so interior writes (offset LEAD + r*66 + 1) are 4-byte
# aligned in bf16 -- required for the DVE 2x_1P perf mode on the h-writes
LEAD = 67
RSZ = LEAD + 36 * PW + 3   # flat elems per partition in R (rows 0..35)
XROWS = 36                 # padded x rows staged in DRAM
X2SZ = 34 * PW             # x im2col buffer elems per partition
NSTEPS = 2 * T
EX_STEPS = (3, 6, 9, 12, 15, 18)   # exchange after these steps

_CACHE = {}


def _rows(s):
    """Computed rows at recurrent step s (1-based): 31 + halo depth."""
    if s >= 19:
        return 31 + (3 if s == 19 else 2)
    return 31 + 3 - ((s - 1) % 3)


def _build_program(use_bf16=True):
    import bass_rust
    from concourse import bacc, mybir, tile

    F32 = mybir.dt.float32
    MMDT = mybir.dt.bfloat16 if use_bf16 else mybir.dt.float32r
    ACT = mybir.ActivationFunctionType
    ALU = mybir.AluOpType

    nc = bacc.Bacc("TRN2", target_bir_lowering=False, debug=False,
                   num_devices=NCORES)

    def din(name, shape, dt=MMDT):
        return nc.dram_tensor(name, shape, dt, kind="ExternalInput").ap()

    xe_d = din("xe", [T, F, XROWS, PW])
    xd_d = din("xd", [T, F, XROWS, PW])
    w_x = {"e": din("w_ex", [128, 256]), "d": din("w_dx", [128, 256])}
    w_p = {ph: [din(f"w_{ph}p{k}", [128, 256]) for k in range(3)]
           for ph in ("e", "d")}
    w_ma = {ph: din(f"w_{ph}ma", [128, 256]) for ph in ("e", "d")}
    w_mb = {ph: din(f"w_{ph}mb", [128, 256]) for ph in ("e", "d")}
    w_op = [din(f"w_op{k}", [128, 8]) for k in range(3)]
    w_oma = din("w_oma", [128, 8])
    w_omb = din("w_omb", [128, 8])
    scl_d = din("scl", [128, 1], F32)  # og tanh scale: 0.5 (o) / 1.0 (g)
    b_m0 = {"e": din("b_e0", [128, 1], F32), "d": din("b_d0", [128, 1], F32)}
    b_m1 = {"e": din("b_e1", [128, 1], F32), "d": din("b_d1", [128, 1], F32)}
    b_o = din("b_o", [8, 1], F32)
    y_d = nc.dram_tensor("y", [T, F, 32, WS], F32, kind="ExternalOutput").ap()

    groups = [[2 * i, 2 * i + 1] for i in range(4)]

    with tile.TileContext(nc) as tc:
        with tc.tile_pool(name="wpool", bufs=1) as wp, \
             tc.tile_pool(name="state", bufs=1) as stp, \
             tc.tile_pool(name="x2p", bufs=2) as x2p, \
             tc.tile_pool(name="gps", bufs=6, space="PSUM") as gps, \
             tc.tile_pool(name="ops", bufs=2, space="PSUM") as ops, \
             tc.tile_pool(name="fip", bufs=3) as fip, \
             tc.tile_pool(name="ogp", bufs=3) as ogp, \
             tc.tile_pool(name="t0p", bufs=3) as t0p, \
             tc.tile_pool(name="t1p", bufs=3) as t1p, \
             tc.tile_pool(name="thp", bufs=3) as thp, \
             tc.tile_pool(name="sxp", bufs=2) as sxp, \
             tc.tile_pool(name="dram", bufs=2, space="DRAM") as dram, \
             tc.tile_pool(name="yyp", bufs=2) as yyp:

            # ---- load weights / biases into SBUF ----
            def wtile(src, shape, tag, dt=MMDT):
                t_ = wp.tile(shape, dt, tag=tag)
                nc.sync.dma_start(t_[:], src[:])
                return t_

            sw_x, sw_p, sw_ma, sw_mb, sb_m0, sb_m1 = {}, {}, {}, {}, {}, {}
            # step-1-critical loads first: sw_x/scl/biases (skip_h step 1
            # needs only these), so x2col(1) isn't stuck behind the full
            # weight set on the sync queue
            sw_x["e"] = wtile(w_x["e"], [128, 256], "wxe")
            sw_op = [wtile(w_op[k], [128, 8], f"wop{k}") for k in range(3)]
            sscl = wtile(scl_d, [128, 1], "scl", F32)
            sb_m0["e"] = wtile(b_m0["e"], [128, 1], "b0e", F32)
            sb_m1["e"] = wtile(b_m1["e"], [128, 1], "b1e", F32)

            # ---- persistent state ----
            rrA = stp.tile([128, RSZ], MMDT, tag="rrA")
            rrB = stp.tile([128, RSZ], MMDT, tag="rrB")
            r2A = stp.tile([128, RSZ], MMDT, tag="r2A")
            r2B = stp.tile([128, RSZ], MMDT, tag="r2B")
            # c in bf16: keeps every epilogue DVE op all-16-bit (2x mode)
            c_t = stp.tile([64, 34 * 64], MMDT, tag="c")

            # PE clock warm-up: sustained matmul activity raises the HAM
            # clock gate before the real work starts.
            for _ in range(32):
                wu = ops.tile([8, 512], F32, tag="pso")
                nc.tensor.matmul(wu[:, 0:256], sw_op[0][:],
                                 sw_x["e"][:, 0:256],
                                 start=True, stop=True)

            def gview(t_, p0, p1, flat_off, nr=8):
                v = t_[p0:p1, flat_off:flat_off + nr * PW]
                v = v.rearrange("p (r c) -> p r c", c=PW)
                return v[:, 0:nr, 0:64]

            x2bufs = [x2p.tile([128, X2SZ], MMDT, tag="x2", name=f"x2{i}")
                      for i in range(2)]
            nc.vector.memset(x2bufs[1][64:128], 0.0)  # step 1 buffer first
            nc.vector.memset(x2bufs[0][64:128], 0.0)

            def emit_x2col(s):
                """Load x im2col for step s: partition (ky*3+kx)*8+ic holds
                the flat padded image shifted by ky*66+kx (contiguous)."""
                ph = "e" if s <= T else "d"
                t_idx = (s - 1) if ph == "e" else (s - 1 - T)
                x_src = xe_d if ph == "e" else xd_d
                ln = (_rows(s) - 1) * PW + 64
                x2 = x2bufs[s % 2]
                flat = x_src[t_idx].rearrange("a r c -> a (r c)")
                for tap in range(9):
                    sh = (tap // 3) * PW + (tap % 3)
                    nc.sync.dma_start(x2[tap * 8:(tap + 1) * 8, 0:ln],
                                      flat[:, sh:sh + ln])
                return x2

            def gate_mms(ps, wp3, wma, wmb, ms, R, R2, r0, nr):
                for k in range(3):
                    nc.tensor.matmul(
                        ps, wp3[k][:, ms],
                        gview(R, 0, 128, LEAD + (r0 + k) * PW, nr),
                        start=False, stop=False)
                nc.tensor.matmul(ps, wma[:, ms],
                                 gview(R2, 0, 128, LEAD + r0 * PW + 1, nr),
                                 start=False, stop=False)
                nc.tensor.matmul(ps, wmb[:, ms],
                                 gview(R, 0, 128, LEAD + (r0 + 2) * PW + 1,
                                       nr),
                                 start=False, stop=True)

            def emit_outconv1(s, R, R2, n2, Y):
                """relu(out conv) for decoder step s, rows 8*n2..8*n2+7,
                written into the step's batched y tile Y."""
                t_o = s - 1 - T
                r0 = n2 * 8
                pso = ops.tile([8, 512], F32, tag="pso")
                for k in range(3):
                    nc.tensor.matmul(pso[:], sw_op[k][:],
                                     gview(R, 0, 128, LEAD + (r0 + k) * PW),
                                     start=(k == 0), stop=False)
                nc.tensor.matmul(pso[:], sw_oma[:, :],
                                 gview(R2, 0, 128, LEAD + r0 * PW + 1),
                                 start=False, stop=False)
                nc.tensor.matmul(pso[:], sw_omb[:, :],
                                 gview(R, 0, 128, LEAD + (r0 + 2) * PW + 1),
                                 start=False, stop=True)
                nc.scalar.activation(Y[:, n2 * 512:(n2 + 1) * 512], pso[:],
                                     ACT.Relu, bias=sb_o[:])
                if n2 == 3:
                    nc.gpsimd.dma_start(
                        y_d[t_o],
                        Y[:].rearrange("p (r c) -> p r c", c=64))

            def gate_x(s, ph, x2v, r0, nr):
                """The x-im2col matmuls: no dependency on the previous
                step's tails, so they issue first and fill PE stalls at
                step boundaries and exchange waits."""
                skip_h = s == 1
                ps0 = gps.tile([128, 512], F32, tag="ps")
                ps1 = gps.tile([128, 512], F32, tag="ps")
                W = nr * 64
                nc.tensor.matmul(ps0[:, 0:W], sw_x[ph][:, 0:128],
                                 x2v[0:128, r0:r0 + nr, 0:64],
                                 start=True, stop=skip_h)
                nc.tensor.matmul(ps1[:, 0:W], sw_x[ph][:, 128:256],
                                 x2v[0:128, r0:r0 + nr, 0:64],
                                 start=True, stop=skip_h)
                return ps0, ps1

            def gate_block(s, ph, R_r, R2_r, R_w, R2_w, pre, r0, nr):
                skip_h = s == 1
                ps0, ps1 = pre
                W = nr * 64
                if not skip_h:
                    gate_mms(ps0[:, 0:W], sw_p[ph], sw_ma[ph],
                             sw_mb[ph], slice(0, 128),
                             R_r, R2_r, r0, nr)
                    gate_mms(ps1[:, 0:W], sw_p[ph], sw_ma[ph],
                             sw_mb[ph], slice(128, 256),
                             R_r, R2_r, r0, nr)

                # epilogue: M0=[f;i] via sigmoid (so the c-chain runs as
                # pure tensor_tensor ops in the DVE 2x mode), M1=[o;g] via
                # tanh with the o fixup folded into the 2x-h convention
                fi = fip.tile([128, 512], MMDT, tag="fi")
                og = ogp.tile([128, 512], MMDT, tag="og")
                nc.scalar.activation(fi[:, 0:W], ps0[:, 0:W], ACT.Sigmoid,
                                     bias=sb_m0[ph][:])
                nc.scalar.activation(og[:, 0:W], ps1[:, 0:W], ACT.Tanh,
                                     bias=sb_m1[ph][:], scale=sscl[:])
                cs = c_t[:, r0 * 64:r0 * 64 + W]
                if skip_h:
                    nc.vector.tensor_tensor(
                        cs, fi[64:128, 0:W], og[64:128, 0:W], ALU.mult)
                else:
                    t0 = t0p.tile([64, 512], MMDT, tag="t0")
                    nc.vector.tensor_tensor(
                        t0[:, 0:W], fi[0:64, 0:W], cs, ALU.mult)
                    t1 = t1p.tile([64, 512], MMDT, tag="t1")
                    nc.vector.tensor_tensor(
                        t1[:, 0:W], fi[64:128, 0:W], og[64:128, 0:W],
                        ALU.mult)
                    nc.vector.tensor_tensor(
                        cs, t0[:, 0:W], t1[:, 0:W], ALU.add)
                return (R_w, R2_w, r0, nr, og)

            def gate_tail(st):
                R_w, R2_w, r0, nr, og = st
                W = nr * 64
                cs = c_t[:, r0 * 64:r0 * 64 + W]
                th = thp.tile([64, 512], MMDT, tag="th")
                nc.scalar.activation(th[:, 0:W], cs, ACT.Tanh)
                thv = th[:, 0:W].rearrange("p (r c) -> p r c", c=64)
                ogv = og[0:64, 0:W].rearrange("p (r c) -> p r c", c=64)
                nc.vector.scalar_tensor_tensor(
                    gview(R_w, 0, 64, LEAD + (r0 + 1) * PW + 1, nr),
                    ogv, 1.0, thv, ALU.add, ALU.mult)
                # shift copies as flat contiguous spans (pad cols are zero,
                # so a uniform address delta realizes the shift; the spill
                # into neighbouring pad columns is never read). Issue cost
                # is spread across three otherwise-idle queues.
                base = LEAD + (r0 + 1) * PW
                n = nr * PW
                src = R_w[0:64, base:base + n]
                nc.sync.dma_start(R_w[64:128, base - 2:base - 2 + n], src)
                nc.scalar.dma_start(R2_w[0:64, base:base + n], src)
                nc.gpsimd.dma_start(R2_w[64:128, base - PW:base - PW + n],
                                    src)

            CCW = 3 * PW + 3 * 64  # h-lower rows 29..31 + c rows 29..31

            def emit_exchange(R_w, R2_w):
                """Pairwise halo exchange: AllReduce h (lower half) and c
                rows 29..31 on 64 partitions, recover partner rows as
                (sum - mine) into ghost rows 32..34 (h) / 32..33 (c).
                The upper-half (col-shifted) ghosts are derived locally."""
                bi = dram.tile([64, CCW], MMDT, tag="ccin")
                bo = dram.tile([64, CCW], MMDT, tag="ccout")
                nc.scalar.dma_start(
                    bi[:, 0:3 * PW],
                    R_w[0:64, LEAD + 30 * PW:LEAD + 33 * PW])
                nc.scalar.dma_start(
                    bi[:, 3 * PW:CCW], c_t[:, 29 * 64:32 * 64])
                nc.gpsimd.collective_compute(
                    "AllReduce", ALU.add, replica_groups=groups,
                    ins=[bi[:].opt()], outs=[bo[:].opt()])
                S = sxp.tile([64, CCW], MMDT, tag="S")
                nc.scalar.dma_start(S[:], bo[:])
                for j in range(3):
                    nc.vector.scalar_tensor_tensor(
                        R_w[0:64, LEAD + (33 + j) * PW:
                            LEAD + (34 + j) * PW],
                        S[:, (2 - j) * PW:(3 - j) * PW], 0.0,
                        R_w[0:64, LEAD + (32 - j) * PW:
                            LEAD + (33 - j) * PW],
                        ALU.add, ALU.subtract)
                for j in range(2):
                    nc.vector.scalar_tensor_tensor(
                        c_t[:, (32 + j) * 64:(33 + j) * 64],
                        S[:, 3 * PW + (2 - j) * 64:
                          3 * PW + (3 - j) * 64], 0.0,
                        c_t[:, (31 - j) * 64:(32 - j) * 64],
                        ALU.add, ALU.subtract)
                src = R_w[0:64, LEAD + 33 * PW:LEAD + 36 * PW]
                nc.scalar.dma_start(
                    R_w[64:128, LEAD + 33 * PW - 2:LEAD + 36 * PW - 2],
                    src)
                nc.scalar.dma_start(
                    R2_w[0:64, LEAD + 33 * PW:LEAD + 36 * PW], src)
                nc.scalar.dma_start(
                    R2_w[64:128, LEAD + 32 * PW:LEAD + 35 * PW], src)

            # warm-up AllReduce: pays the NRT first-collective setup cost
            # (~20us) during the PE warm-up instead of at step 3
            wbi = dram.tile([64, 8], MMDT, tag="wcc")
            wbo = dram.tile([64, 8], MMDT, tag="wcc2")
            nc.scalar.dma_start(wbi[:], sw_op[0][0:64, 0:8])
            nc.gpsimd.collective_compute(
                "AllReduce", ALU.add, replica_groups=groups,
                ins=[wbi[:].opt()], outs=[wbo[:].opt()])

            pend = None           # (tile_id, tail_state)
            cur_Y = None
            x2_cur = emit_x2col(1)
            nc.gpsimd.memset(rrB[:], 0.0)
            nc.gpsimd.memset(r2B[:], 0.0)
            nc.vector.memset(rrA[:], 0.0)
            nc.vector.memset(r2A[:], 0.0)
            # remaining weights, behind the step-1 x2col on the sync queue
            sw_oma = wtile(w_oma, [128, 8], "woma")
            sw_omb = wtile(w_omb, [128, 8], "womb")
            sb_o = wtile(b_o, [8, 1], "bo", F32)
            sw_p["e"] = [wtile(w_p["e"][k], [128, 256], f"wpe{k}")
                         for k in range(3)]
            sw_ma["e"] = wtile(w_ma["e"], [128, 256], "wmae")
            sw_mb["e"] = wtile(w_mb["e"], [128, 256], "wmbe")
            sw_x["d"] = wtile(w_x["d"], [128, 256], "wxd")
            sw_p["d"] = [wtile(w_p["d"][k], [128, 256], f"wpd{k}")
                         for k in range(3)]
            sw_ma["d"] = wtile(w_ma["d"], [128, 256], "wmad")
            sw_mb["d"] = wtile(w_mb["d"], [128, 256], "wmbd")
            sb_m0["d"] = wtile(b_m0["d"], [128, 1], "b0d", F32)
            sb_m1["d"] = wtile(b_m1["d"], [128, 1], "b1d", F32)
            for s in range(1, NSTEPS + 1):
                ph = "e" if s <= T else "d"
                rows = _rows(s)
                tiles = [(0, 8), (8, 8), (16, 8), (24, 8)]
                if rows > 32:
                    tiles.append((32, rows - 32))
                send = s in EX_STEPS
                order = [0, 1, 3, 2] if send else list(range(len(tiles)))
                if s % 2 == 0:
                    R_r, R_w, R2_r, R2_w = rrA, rrB, r2A, r2B
                else:
                    R_r, R_w, R2_r, R2_w = rrB, rrA, r2B, r2A

                x2v = x2_cur[:].rearrange("p (r c) -> p r c", c=PW)
                if s < NSTEPS:
                    x2_next = emit_x2col(s + 1)

                for i, n in enumerate(order):
                    r0, nr = tiles[n]
                    # x matmuls first, then the prev decoder step's out
                    # conv: both independent of this step's tail chain, so
                    # they fill PE stalls at boundaries and ghost waits
                    pre = gate_x(s, ph, x2v, r0, nr)
                    if s > T + 1 and i < 4:
                        if i == 0:
                            cur_Y = yyp.tile([8, 2048], F32, tag="Y")
                        emit_outconv1(s - 1, R_r, R2_r, i, cur_Y)
                    st = gate_block(s, ph, R_r, R2_r, R_w, R2_w, pre,
                                    r0, nr)
                    if pend is not None:
                        gate_tail(pend[1])
                        if send and pend[0] == 3:
                            emit_exchange(R_w, R2_w)
                    pend = (n, st)
                    # final step's out conv rows 0:24 only need tails 0-3
                    if s == NSTEPS and i == 4:
                        fin_Y = yyp.tile([8, 2048], F32, tag="Y")
                        for _n2 in range(3):
                            emit_outconv1(NSTEPS, R_w, R2_w, _n2, fin_Y)

                if pend is not None:
                    gate_tail(pend[1])
                    if send and pend[0] == 3:
                        emit_exchange(R_w, R2_w)
                    pend = None

                if s < NSTEPS:
                    x2_cur = x2_next

            emit_outconv1(NSTEPS, rrB, r2B, 3, fin_Y)

    nc.compile()
    return nc


def _prep_core_inputs(core, enc_in, dec_in, enc_W, enc_b, dec_W, dec_b,
                      out_W, out_b, use_bf16=True):
    import ml_dtypes
    mm_np = ml_dtypes.bfloat16 if use_bf16 else np.float32
    b, half = core // 2, core % 2
    # gate permutation: [f, i, o, g]
    perm = np.concatenate([np.arange(0, 128), np.arange(192, 256),
                           np.arange(128, 192)])

    def prep_x(x):
        x = x[b]  # [T, F, 64, 64]
        if half:
            x = x[:, :, ::-1, :]
        xp = np.zeros((T, F, XROWS, PW), np.float32)
        xp[:, :, 1:36, 1:65] = x[:, :, 0:35, :]
        return np.ascontiguousarray(xp)

    def prep_gateW(W, bias):
        Wf = W[:, :, ::-1, :] if half else W
        Wp = np.ascontiguousarray(Wf[perm])  # [256, 72, 3, 3]
        bp = bias[perm].astype(np.float32)
        lx = np.zeros((128, 256), np.float32)
        lx[0:72] = Wp[:, :F].transpose(2, 3, 1, 0).reshape(72, 256)
        # h-tap weights halved: stored h carries a 2x factor
        lp = [0.5 * np.concatenate(
            [Wp[:, F:, k, 0].T, Wp[:, F:, k, 2].T], axis=0)
            for k in range(3)]
        lma = 0.5 * np.concatenate([Wp[:, F:, 0, 1].T, Wp[:, F:, 1, 1].T],
                                   axis=0)
        lmb = np.zeros((128, 256), np.float32)
        lmb[0:64] = 0.5 * Wp[:, F:, 2, 1].T
        b0 = bp[0:128].copy()  # f,i biases plain: ACT0 is a true sigmoid
        b1 = np.concatenate([0.5 * bp[128:192], bp[192:256]])
        return (np.ascontiguousarray(lx),
                [np.ascontiguousarray(a) for a in lp],
                np.ascontiguousarray(lma), lmb,
                np.ascontiguousarray(b0.reshape(128, 1)),
                np.ascontiguousarray(b1.reshape(128, 1)))

    ex, ep, ema, emb, eb0, eb1 = prep_gateW(enc_W, enc_b)
    dx, dp, dma_, dmb, db0, db1 = prep_gateW(dec_W, dec_b)
    oWf = out_W[:, :, ::-1, :] if half else out_W
    op = [np.ascontiguousarray(0.5 * np.concatenate(
        [oWf[:, :, k, 0].T, oWf[:, :, k, 2].T], axis=0).astype(np.float32))
        for k in range(3)]
    oma = np.ascontiguousarray(0.5 * np.concatenate(
        [oWf[:, :, 0, 1].T, oWf[:, :, 1, 1].T], axis=0))
    omb = np.zeros((128, 8), np.float32)
    omb[0:64] = 0.5 * oWf[:, :, 2, 1].T
    scl = np.concatenate([np.full(64, 0.5, np.float32),
                          np.full(64, 1.0, np.float32)]).reshape(128, 1)

    m = {"xe": prep_x(enc_in), "xd": prep_x(dec_in),
         "w_ex": ex, "w_dx": dx,
         "w_ema": ema, "w_emb": emb, "w_dma": dma_, "w_dmb": dmb,
         "w_oma": oma, "w_omb": omb, "scl": scl,
         "b_e0": eb0, "b_e1": eb1, "b_d0": db0, "b_d1": db1,
         "b_o": np.ascontiguousarray(out_b.reshape(8, 1).astype(np.float32))}
    for k in range(3):
        m[f"w_ep{k}"] = ep[k]
        m[f"w_dp{k}"] = dp[k]
        m[f"w_op{k}"] = op[k]
    f32_keys = {"b_e0", "b_e1", "b_d0", "b_d1", "b_o", "scl"}
    return {k: np.ascontiguousarray(np.asarray(
        v, np.float32 if k in f32_keys else mm_np)) for k, v in m.items()}


def _install_trace_hook():
    """Shim antenv.axon_hooks for NTFF profiling (dev only)."""
    import contextlib
    import ctypes
    import types

    so = "/opt/axon/libaxon_pjrt.so"
    if "antenv.axon_hooks" in sys.modules or not os.path.exists(so):
        return
    lib = ctypes.CDLL(so)
    if not hasattr(lib, "axon_start_nrt_profile"):
        return
    lib.axon_start_nrt_profile.argtypes = [ctypes.POINTER(ctypes.c_int64),
                                           ctypes.c_size_t]
    lib.axon_start_nrt_profile.restype = ctypes.c_int64
    lib.axon_stop_nrt_profile.argtypes = [ctypes.c_char_p]
    lib.axon_stop_nrt_profile.restype = ctypes.c_int64

    def _mk():
        @contextlib.contextmanager
        def _hook(output_dir, device_ids):
            import jax
            jax.devices()
            if device_ids:
                ids = (ctypes.c_int64 * len(device_ids))(*device_ids)
                rc = lib.axon_start_nrt_profile(ids, len(device_ids))
            else:
                rc = lib.axon_start_nrt_profile(None, 0)
            if rc != 0:
                raise RuntimeError(f"axon_start_nrt_profile rc={rc}")
            try:
                yield
            finally:
                lib.axon_stop_nrt_profile(str(output_dir).encode())
        return _hook

    mod = types.ModuleType("antenv.axon_hooks")
    mod.get_axon_ntff_profile_hook = _mk
    sys.modules["antenv.axon_hooks"] = mod


def kernel(enc_in, dec_in, enc_W, enc_b, dec_W, dec_b, out_W, out_b):
    from concourse.bass_utils import run_bass_kernel_spmd

    trace = os.environ.get("KERNEL_TRACE", "") == "1"
    if trace:
        _install_trace_hook()

    use_bf16 = os.environ.get("KERNEL_DTYPE", "bf16") != "f32r"
    if "nc" not in _CACHE:
        _CACHE["nc"] = _build_program(use_bf16)
    nc = _CACHE["nc"]

    args = (np.asarray(enc_in, np.float32), np.asarray(dec_in, np.float32),
            np.asarray(enc_W, np.float32), np.asarray(enc_b, np.float32),
            np.asarray(dec_W, np.float32), np.asarray(dec_b, np.float32),
            np.asarray(out_W, np.float32), np.asarray(out_b, np.float32))
    in_maps = [_prep_core_inputs(c, *args, use_bf16=use_bf16)
               for c in range(NCORES)]

    res = run_bass_kernel_spmd(nc, in_maps, list(range(NCORES)), trace=trace)
    if trace:
        _CACHE["exec_time_ns"] = res.exec_time_ns

    B = enc_in.shape[0]
    out = np.empty((B, T, F, HS, WS), np.float32)
    for c in range(NCORES):
        b, half = c // 2, c % 2
        yc = res.results[c]["y"]  # [T, F, 32, 64]
        if half:
            out[b, :, :, 32:64, :] = yc[:, :, ::-1, :]
        else:
            out[b, :, :, 0:32, :] = yc
    return out


# revision 26
# speedup vs baseline: 1.1553x; 1.1553x over previous
"""EncDec ConvLSTM kernel for 8 Trainium2 NeuronCores.

Sharding: 8 cores = 4 (batch) x 2 (spatial row-halves). Each core owns 32
output rows; a 3-row halo is refreshed by a pairwise AllReduce exchange
every 3rd step, so per-step redundant compute shrinks from the old
no-comms scheme's 880 rows/core to 661. Row-half 1 cores receive a
vertically flipped image and ky-flipped conv weights, so a single SPMD
program serves all cores. Ghost rows are recovered rank-agnostically as
(pair-sum - mine); the exchange is issued right after the boundary tile
(computed first on exchange steps) so its ~13us latency hides under ~2.5
steps of compute.

Conv3x3 maps to PE matmuls over pixels (N up to 512, bf16), all with
full K=128 stationary loads so LDWEIGHTS hides behind in-flight matmuls.
Two double-buffered state tiles per step:
  R  = [h (parts 0:64) | h col-shifted +2 (parts 64:128)]
  R2 = [h (parts 0:64) | h row-shifted +1 (parts 64:128)]
Per row-tile and M-tile, 6 matmuls: x-im2col (K=72 zero-padded to 128),
3 paired-kx taps on R at row offsets 0/1/2, middle-column ky=0/1 as one
K=128 MM on R2, and ky=2 zero-padded on R. The three shift copies per
tile are flat contiguous-span DMAs (pad columns are zero, so a uniform
address delta realizes the shift), which issue far cheaper than strided
row views. The per-step x im2col loads are 3 batched DMAs (one per ky)
using a custom overlapping-window access pattern.

Epilogue runs entirely in tanh form (sigmoid(z) = 0.5*tanh(z/2)+0.5 with
the /2 folded into ACT scale/bias and the +1/x0.5 fixups into
scalar_tensor_tensor ops; stored h and c carry a 2x factor compensated
by pre-halved h-tap weights), 3 ACT ops per tile. The (f+1)*c product
runs on gpsimd to balance DVE load. Each tile's tail (tanh(c), h write,
shifts) is emitted one tile later so the ACT FIFO never head-of-line
blocks on the DVE c-update chain; decoder out-convs are interleaved
between gate tiles and their outputs batched to one DMA per step. A
32-matmul warm-up raises the HAM clock gate before the real work.
"""

import os
import sys

import numpy as np

for _p in ("/opt/trn_rl_repo", "/root/.axon_site/_ro/trn_rl_repo"):
    if os.path.isdir(_p) and _p not in sys.path:
        sys.path.append(_p)

T = 10
F = 8
HD = 64
HS = 64
WS = 64
NCORES = 8
PW = 66   # padded grid width
# LEAD is odd so interior writes (offset LEAD + r*66 + 1) are 4-byte
# aligned in bf16 -- required for the DVE 2x_1P perf mode on the h-writes
LEAD = 67
RSZ = LEAD + 36 * PW + 3   # flat elems per partition in R (rows 0..35)
XROWS = 36                 # padded x rows staged in DRAM
X2SZ = 34 * PW             # x im2col buffer elems per partition
NSTEPS = 2 * T
EX_STEPS = (3, 6, 9, 12, 15, 18)   # exchange after these steps

_CACHE = {}


def _rows(s):
    """Computed rows at recurrent step s (1-based): 31 + halo depth."""
    if s >= 19:
        return 31 + (3 if s == 19 else 2)
    return 31 + 3 - ((s - 1) % 3)


def _build_program(use_bf16=True):
    import bass_rust
    from concourse import bacc, mybir, tile

    F32 = mybir.dt.float32
    MMDT = mybir.dt.bfloat16 if use_bf16 else mybir.dt.float32r
    ACT = mybir.ActivationFunctionType
    ALU = mybir.AluOpType

    nc = bacc.Bacc("TRN2", target_bir_lowering=False, debug=False,
                   num_devices=NCORES)

    def din(name, shape, dt=MMDT):
        return nc.dram_tensor(name, shape, dt, kind="ExternalInput").ap()

    xe_d = din("xe", [T, F, XROWS, PW])
    xd_d = din("xd", [T, F, XROWS, PW])
    w_x = {"e": din("w_ex", [128, 256]), "d": din("w_dx", [128, 256])}
    w_p = {ph: [din(f"w_{ph}p{k}", [128, 256]) for k in range(3)]
           for ph in ("e", "d")}
    w_ma = {ph: din(f"w_{ph}ma", [128, 256]) for ph in ("e", "d")}
    w_mb = {ph: din(f"w_{ph}mb", [128, 256]) for ph in ("e", "d")}
    w_op = [din(f"w_op{k}", [128, 8]) for k in range(3)]
    w_oma = din("w_oma", [128, 8])
    w_omb = din("w_omb", [128, 8])
    scl_d = din("scl", [128, 1], F32)  # og tanh scale: 0.5 (o) / 1.0 (g)
    b_m0 = {"e": din("b_e0", [128, 1], F32), "d": din("b_d0", [128, 1], F32)}
    b_m1 = {"e": din("b_e1", [128, 1], F32), "d": din("b_d1", [128, 1], F32)}
    b_o = din("b_o", [8, 1], F32)
    y_d = nc.dram_tensor("y", [T, F, 32, WS], F32, kind="ExternalOutput").ap()

    groups = [[2 * i, 2 * i + 1] for i in range(4)]

    with tile.TileContext(nc) as tc:
        with tc.tile_pool(name="wpool", bufs=1) as wp, \
             tc.tile_pool(name="state", bufs=1) as stp, \
             tc.tile_pool(name="x2p", bufs=2) as x2p, \
             tc.tile_pool(name="gps", bufs=6, space="PSUM") as gps, \
             tc.tile_pool(name="ops", bufs=2, space="PSUM") as ops, \
             tc.tile_pool(name="fip", bufs=3) as fip, \
             tc.tile_pool(name="ogp", bufs=3) as ogp, \
             tc.tile_pool(name="t0p", bufs=3) as t0p, \
             tc.tile_pool(name="t1p", bufs=3) as t1p, \
             tc.tile_pool(name="thp", bufs=3) as thp, \
             tc.tile_pool(name="sxp", bufs=2) as sxp, \
             tc.tile_pool(name="dram", bufs=2, space="DRAM") as dram, \
             tc.tile_pool(name="yyp", bufs=2) as yyp:

            # ---- load weights / biases into SBUF ----
            def wtile(src, shape, tag, dt=MMDT):
                t_ = wp.tile(shape, dt, tag=tag)
                nc.sync.dma_start(t_[:], src[:])
                return t_

            sw_x, sw_p, sw_ma, sw_mb, sb_m0, sb_m1 = {}, {}, {}, {}, {}, {}
            # step-1-critical loads first: sw_x/scl/biases (skip_h step 1
            # needs only these), so x2col(1) isn't stuck behind the full
            # weight set on the sync queue
            sw_x["e"] = wtile(w_x["e"], [128, 256], "wxe")
            sw_op = [wtile(w_op[k], [128, 8], f"wop{k}") for k in range(3)]
            sscl = wtile(scl_d, [128, 1], "scl", F32)
            sb_m0["e"] = wtile(b_m0["e"], [128, 1], "b0e", F32)
            sb_m1["e"] = wtile(b_m1["e"], [128, 1], "b1e", F32)
            sw_p["e"] = [wtile(w_p["e"][k], [128, 256], f"wpe{k}")
                         for k in range(3)]
            sw_ma["e"] = wtile(w_ma["e"], [128, 256], "wmae")
            sw_mb["e"] = wtile(w_mb["e"], [128, 256], "wmbe")

            # ---- persistent state ----
            rrA = stp.tile([128, RSZ], MMDT, tag="rrA")
            rrB = stp.tile([128, RSZ], MMDT, tag="rrB")
            r2A = stp.tile([128, RSZ], MMDT, tag="r2A")
            r2B = stp.tile([128, RSZ], MMDT, tag="r2B")
            # c in bf16: keeps every epilogue DVE op all-16-bit (2x mode)
            c_t = stp.tile([64, 34 * 64], MMDT, tag="c")

            # PE clock warm-up: sustained matmul activity raises the HAM
            # clock gate before the real work starts.
            for _ in range(32):
                wu = ops.tile([8, 512], F32, tag="pso")
                nc.tensor.matmul(wu[:, 0:256], sw_op[0][:],
                                 sw_x["e"][:, 0:256],
                                 start=True, stop=True)

            def gview(t_, p0, p1, flat_off, nr=8):
                v = t_[p0:p1, flat_off:flat_off + nr * PW]
                v = v.rearrange("p (r c) -> p r c", c=PW)
                return v[:, 0:nr, 0:64]

            x2bufs = [x2p.tile([128, X2SZ], MMDT, tag="x2", name=f"x2{i}")
                      for i in range(2)]
            nc.vector.memset(x2bufs[1][64:128], 0.0)  # step 1 buffer first
            nc.vector.memset(x2bufs[0][64:128], 0.0)

            def emit_x2col(s):
                """Load x im2col for step s: partition (ky*3+kx)*8+ic holds
                the flat padded image shifted by ky*66+kx (contiguous)."""
                ph = "e" if s <= T else "d"
                t_idx = (s - 1) if ph == "e" else (s - 1 - T)
                x_src = xe_d if ph == "e" else xd_d
                ln = (_rows(s) - 1) * PW + 64
                x2 = x2bufs[s % 2]
                flat = x_src[t_idx].rearrange("a r c -> a (r c)")
                for tap in range(9):
                    sh = (tap // 3) * PW + (tap % 3)
                    nc.sync.dma_start(x2[tap * 8:(tap + 1) * 8, 0:ln],
                                      flat[:, sh:sh + ln])
                return x2

            def gate_mms(ps, wp3, wma, wmb, ms, R, R2, r0, nr):
                for k in range(3):
                    nc.tensor.matmul(
                        ps, wp3[k][:, ms],
                        gview(R, 0, 128, LEAD + (r0 + k) * PW, nr),
                        start=False, stop=False)
                nc.tensor.matmul(ps, wma[:, ms],
                                 gview(R2, 0, 128, LEAD + r0 * PW + 1, nr),
                                 start=False, stop=False)
                nc.tensor.matmul(ps, wmb[:, ms],
                                 gview(R, 0, 128, LEAD + (r0 + 2) * PW + 1,
                                       nr),
                                 start=False, stop=True)

            def emit_outconv1(s, R, R2, n2, Y):
                """relu(out conv) for decoder step s, rows 8*n2..8*n2+7,
                written into the step's batched y tile Y."""
                t_o = s - 1 - T
                r0 = n2 * 8
                pso = ops.tile([8, 512], F32, tag="pso")
                for k in range(3):
                    nc.tensor.matmul(pso[:], sw_op[k][:],
                                     gview(R, 0, 128, LEAD + (r0 + k) * PW),
                                     start=(k == 0), stop=False)
                nc.tensor.matmul(pso[:], sw_oma[:, :],
                                 gview(R2, 0, 128, LEAD + r0 * PW + 1),
                                 start=False, stop=False)
                nc.tensor.matmul(pso[:], sw_omb[:, :],
                                 gview(R, 0, 128, LEAD + (r0 + 2) * PW + 1),
                                 start=False, stop=True)
                nc.scalar.activation(Y[:, n2 * 512:(n2 + 1) * 512], pso[:],
                                     ACT.Relu, bias=sb_o[:])
                if n2 == 3:
                    nc.gpsimd.dma_start(
                        y_d[t_o],
                        Y[:].rearrange("p (r c) -> p r c", c=64))

            def gate_x(s, ph, x2v, r0, nr):
                """The x-im2col matmuls: no dependency on the previous
                step's tails, so they issue first and fill PE stalls at
                step boundaries and exchange waits."""
                skip_h = s == 1
                ps0 = gps.tile([128, 512], F32, tag="ps")
                ps1 = gps.tile([128, 512], F32, tag="ps")
                W = nr * 64
                nc.tensor.matmul(ps0[:, 0:W], sw_x[ph][:, 0:128],
                                 x2v[0:128, r0:r0 + nr, 0:64],
                                 start=True, stop=skip_h)
                nc.tensor.matmul(ps1[:, 0:W], sw_x[ph][:, 128:256],
                                 x2v[0:128, r0:r0 + nr, 0:64],
                                 start=True, stop=skip_h)
                return ps0, ps1

            def gate_block(s, ph, R_r, R2_r, R_w, R2_w, pre, r0, nr):
                skip_h = s == 1
                ps0, ps1 = pre
                W = nr * 64
                if not skip_h:
                    gate_mms(ps0[:, 0:W], sw_p[ph], sw_ma[ph],
                             sw_mb[ph], slice(0, 128),
                             R_r, R2_r, r0, nr)
                    gate_mms(ps1[:, 0:W], sw_p[ph], sw_ma[ph],
                             sw_mb[ph], slice(128, 256),
                             R_r, R2_r, r0, nr)

                # epilogue: M0=[f;i] via sigmoid (so the c-chain runs as
                # pure tensor_tensor ops in the DVE 2x mode), M1=[o;g] via
                # tanh with the o fixup folded into the 2x-h convention
                fi = fip.tile([128, 512], MMDT, tag="fi")
                og = ogp.tile([128, 512], MMDT, tag="og")
                nc.scalar.activation(fi[:, 0:W], ps0[:, 0:W], ACT.Sigmoid,
                                     bias=sb_m0[ph][:])
                nc.scalar.activation(og[:, 0:W], ps1[:, 0:W], ACT.Tanh,
                                     bias=sb_m1[ph][:], scale=sscl[:])
                cs = c_t[:, r0 * 64:r0 * 64 + W]
                if skip_h:
                    nc.vector.tensor_tensor(
                        cs, fi[64:128, 0:W], og[64:128, 0:W], ALU.mult)
                else:
                    t0 = t0p.tile([64, 512], MMDT, tag="t0")
                    nc.vector.tensor_tensor(
                        t0[:, 0:W], fi[0:64, 0:W], cs, ALU.mult)
                    t1 = t1p.tile([64, 512], MMDT, tag="t1")
                    nc.vector.tensor_tensor(
                        t1[:, 0:W], fi[64:128, 0:W], og[64:128, 0:W],
                        ALU.mult)
                    nc.vector.tensor_tensor(
                        cs, t0[:, 0:W], t1[:, 0:W], ALU.add)
                return (R_w, R2_w, r0, nr, og)

            def gate_tail(st):
                R_w, R2_w, r0, nr, og = st
                W = nr * 64
                cs = c_t[:, r0 * 64:r0 * 64 + W]
                th = thp.tile([64, 512], MMDT, tag="th")
                nc.scalar.activation(th[:, 0:W], cs, ACT.Tanh)
                thv = th[:, 0:W].rearrange("p (r c) -> p r c", c=64)
                ogv = og[0:64, 0:W].rearrange("p (r c) -> p r c", c=64)
                nc.vector.scalar_tensor_tensor(
                    gview(R_w, 0, 64, LEAD + (r0 + 1) * PW + 1, nr),
                    ogv, 1.0, thv, ALU.add, ALU.mult)
                # shift copies as flat contiguous spans (pad cols are zero,
                # so a uniform address delta realizes the shift; the spill
                # into neighbouring pad columns is never read). Issue cost
                # is spread across three otherwise-idle queues.
                base = LEAD + (r0 + 1) * PW
                n = nr * PW
                src = R_w[0:64, base:base + n]
                nc.sync.dma_start(R_w[64:128, base - 2:base - 2 + n], src)
                nc.scalar.dma_start(R2_w[0:64, base:base + n], src)
                nc.gpsimd.dma_start(R2_w[64:128, base - PW:base - PW + n],
                                    src)

            CCW = 3 * PW + 3 * 64  # h-lower rows 29..31 + c rows 29..31

            def emit_exchange(R_w, R2_w):
                """Pairwise halo exchange: AllReduce h (lower half) and c
                rows 29..31 on 64 partitions, recover partner rows as
                (sum - mine) into ghost rows 32..34 (h) / 32..33 (c).
                The upper-half (col-shifted) ghosts are derived locally."""
                bi = dram.tile([64, CCW], MMDT, tag="ccin")
                bo = dram.tile([64, CCW], MMDT, tag="ccout")
                nc.scalar.dma_start(
                    bi[:, 0:3 * PW],
                    R_w[0:64, LEAD + 30 * PW:LEAD + 33 * PW])
                nc.scalar.dma_start(
                    bi[:, 3 * PW:CCW], c_t[:, 29 * 64:32 * 64])
                nc.gpsimd.collective_compute(
                    "AllReduce", ALU.add, replica_groups=groups,
                    ins=[bi[:].opt()], outs=[bo[:].opt()])
                S = sxp.tile([64, CCW], MMDT, tag="S")
                nc.scalar.dma_start(S[:], bo[:])
                for j in range(3):
                    nc.vector.scalar_tensor_tensor(
                        R_w[0:64, LEAD + (33 + j) * PW:
                            LEAD + (34 + j) * PW],
                        S[:, (2 - j) * PW:(3 - j) * PW], 0.0,
                        R_w[0:64, LEAD + (32 - j) * PW:
                            LEAD + (33 - j) * PW],
                        ALU.add, ALU.subtract)
                for j in range(2):
                    nc.vector.scalar_tensor_tensor(
                        c_t[:, (32 + j) * 64:(33 + j) * 64],
                        S[:, 3 * PW + (2 - j) * 64:
                          3 * PW + (3 - j) * 64], 0.0,
                        c_t[:, (31 - j) * 64:(32 - j) * 64],
                        ALU.add, ALU.subtract)
                src = R_w[0:64, LEAD + 33 * PW:LEAD + 36 * PW]
                nc.scalar.dma_start(
                    R_w[64:128, LEAD + 33 * PW - 2:LEAD + 36 * PW - 2],
                    src)
                nc.scalar.dma_start(
                    R2_w[0:64, LEAD + 33 * PW:LEAD + 36 * PW], src)
                nc.scalar.dma_start(
                    R2_w[64:128, LEAD + 32 * PW:LEAD + 35 * PW], src)

            # warm-up AllReduce: pays the NRT first-collective setup cost
            # (~20us) during the PE warm-up instead of at step 3
            wbi = dram.tile([64, 8], MMDT, tag="wcc")
            wbo = dram.tile([64, 8], MMDT, tag="wcc2")
            nc.scalar.dma_start(wbi[:], sw_op[0][0:64, 0:8])
            nc.gpsimd.collective_compute(
                "AllReduce", ALU.add, replica_groups=groups,
                ins=[wbi[:].opt()], outs=[wbo[:].opt()])

            pend = None           # (tile_id, tail_state)
            cur_Y = None
            x2_cur = emit_x2col(1)
            x2_nxt = emit_x2col(2)
            nc.gpsimd.memset(rrB[:], 0.0)
            nc.gpsimd.memset(r2B[:], 0.0)
            nc.vector.memset(rrA[:], 0.0)
            nc.vector.memset(r2A[:], 0.0)
            # remaining weights, behind the early x2cols on the sync queue
            sw_oma = wtile(w_oma, [128, 8], "woma")
            sw_omb = wtile(w_omb, [128, 8], "womb")
            sb_o = wtile(b_o, [8, 1], "bo", F32)
            sw_x["d"] = wtile(w_x["d"], [128, 256], "wxd")
            sw_p["d"] = [wtile(w_p["d"][k], [128, 256], f"wpd{k}")
                         for k in range(3)]
            sw_ma["d"] = wtile(w_ma["d"], [128, 256], "wmad")
            sw_mb["d"] = wtile(w_mb["d"], [128, 256], "wmbd")
            sb_m0["d"] = wtile(b_m0["d"], [128, 1], "b0d", F32)
            sb_m1["d"] = wtile(b_m1["d"], [128, 1], "b1d", F32)
            for s in range(1, NSTEPS + 1):
                ph = "e" if s <= T else "d"
                rows = _rows(s)
                tiles = [(0, 8), (8, 8), (16, 8), (24, 8)]
                if rows > 32:
                    tiles.append((32, rows - 32))
                send = s in EX_STEPS
                order = [0, 3, 1, 2] if send else list(range(len(tiles)))
                if s % 2 == 0:
                    R_r, R_w, R2_r, R2_w = rrA, rrB, r2A, r2B
                else:
                    R_r, R_w, R2_r, R2_w = rrB, rrA, r2B, r2A

                x2v = x2_cur[:].rearrange("p (r c) -> p r c", c=PW)
                if s == 1:
                    x2_next = x2_nxt  # prefetched before the weight bulk
                elif s < NSTEPS:
                    x2_next = emit_x2col(s + 1)

                for i, n in enumerate(order):
                    r0, nr = tiles[n]
                    # x matmuls first, then the prev decoder step's out
                    # conv: both independent of this step's tail chain, so
                    # they fill PE stalls at boundaries and ghost waits
                    pre = gate_x(s, ph, x2v, r0, nr)
                    if s > T + 1 and i < 4:
                        if i == 0:
                            cur_Y = yyp.tile([8, 2048], F32, tag="Y")
                        emit_outconv1(s - 1, R_r, R2_r, i, cur_Y)
                    st = gate_block(s, ph, R_r, R2_r, R_w, R2_w, pre,
                                    r0, nr)
                    if pend is not None:
                        gate_tail(pend[1])
                        if send and pend[0] == 3:
                            emit_exchange(R_w, R2_w)
                    pend = (n, st)
                    # final step's out conv rows 0:24 only need tails 0-3
                    if s == NSTEPS and i == 4:
                        fin_Y = yyp.tile([8, 2048], F32, tag="Y")
                        for _n2 in range(3):
                            emit_outconv1(NSTEPS, R_w, R2_w, _n2, fin_Y)

                if pend is not None:
                    gate_tail(pend[1])
                    if send and pend[0] == 3:
                        emit_exchange(R_w, R2_w)
                    pend = None

                if s < NSTEPS:
                    x2_cur = x2_next

            emit_outconv1(NSTEPS, rrB, r2B, 3, fin_Y)

    nc.compile()
    return nc


def _prep_core_inputs(core, enc_in, dec_in, enc_W, enc_b, dec_W, dec_b,
                      out_W, out_b, use_bf16=True):
    import ml_dtypes
    mm_np = ml_dtypes.bfloat16 if use_bf16 else np.float32
    b, half = core // 2, core % 2
    # gate permutation: [f, i, o, g]
    perm = np.concatenate([np.arange(0, 128), np.arange(192, 256),
                           np.arange(128, 192)])

    def prep_x(x):
        x = x[b]  # [T, F, 64, 64]
        if half:
            x = x[:, :, ::-1, :]
        xp = np.zeros((T, F, XROWS, PW), np.float32)
        xp[:, :, 1:36, 1:65] = x[:, :, 0:35, :]
        return np.ascontiguousarray(xp)

    def prep_gateW(W, bias):
        Wf = W[:, :, ::-1, :] if half else W
        Wp = np.ascontiguousarray(Wf[perm])  # [256, 72, 3, 3]
        bp = bias[perm].astype(np.float32)
        lx = np.zeros((128, 256), np.float32)
        lx[0:72] = Wp[:, :F].transpose(2, 3, 1, 0).reshape(72, 256)
        # h-tap weights halved: stored h carries a 2x factor
        lp = [0.5 * np.concatenate(
            [Wp[:, F:, k, 0].T, Wp[:, F:, k, 2].T], axis=0)
            for k in range(3)]
        lma = 0.5 * np.concatenate([Wp[:, F:, 0, 1].T, Wp[:, F:, 1, 1].T],
                                   axis=0)
        lmb = np.zeros((128, 256), np.float32)
        lmb[0:64] = 0.5 * Wp[:, F:, 2, 1].T
        b0 = bp[0:128].copy()  # f,i biases plain: ACT0 is a true sigmoid
        b1 = np.concatenate([0.5 * bp[128:192], bp[192:256]])
        return (np.ascontiguousarray(lx),
                [np.ascontiguousarray(a) for a in lp],
                np.ascontiguousarray(lma), lmb,
                np.ascontiguousarray(b0.reshape(128, 1)),
                np.ascontiguousarray(b1.reshape(128, 1)))

    ex, ep, ema, emb, eb0, eb1 = prep_gateW(enc_W, enc_b)
    dx, dp, dma_, dmb, db0, db1 = prep_gateW(dec_W, dec_b)
    oWf = out_W[:, :, ::-1, :] if half else out_W
    op = [np.ascontiguousarray(0.5 * np.concatenate(
        [oWf[:, :, k, 0].T, oWf[:, :, k, 2].T], axis=0).astype(np.float32))
        for k in range(3)]
    oma = np.ascontiguousarray(0.5 * np.concatenate(
        [oWf[:, :, 0, 1].T, oWf[:, :, 1, 1].T], axis=0))
    omb = np.zeros((128, 8), np.float32)
    omb[0:64] = 0.5 * oWf[:, :, 2, 1].T
    scl = np.concatenate([np.full(64, 0.5, np.float32),
                          np.full(64, 1.0, np.float32)]).reshape(128, 1)

    m = {"xe": prep_x(enc_in), "xd": prep_x(dec_in),
         "w_ex": ex, "w_dx": dx,
         "w_ema": ema, "w_emb": emb, "w_dma": dma_, "w_dmb": dmb,
         "w_oma": oma, "w_omb": omb, "scl": scl,
         "b_e0": eb0, "b_e1": eb1, "b_d0": db0, "b_d1": db1,
         "b_o": np.ascontiguousarray(out_b.reshape(8, 1).astype(np.float32))}
    for k in range(3):
        m[f"w_ep{k}"] = ep[k]
        m[f"w_dp{k}"] = dp[k]
        m[f"w_op{k}"] = op[k]
    f32_keys = {"b_e0", "b_e1", "b_d0", "b_d1", "b_o", "scl"}
    return {k: np.ascontiguousarray(np.asarray(
        v, np.float32 if k in f32_keys else mm_np)) for k, v in m.items()}


def _install_trace_hook():
    """Shim antenv.axon_hooks for NTFF profiling (dev only)."""
    import contextlib
    import ctypes
    import types

    so = "/opt/axon/libaxon_pjrt.so"
    if "antenv.axon_hooks" in sys.modules or not os.path.exists(so):
        return
    lib = ctypes.CDLL(so)
    if not hasattr(lib, "axon_start_nrt_profile"):
        return
    lib.axon_start_nrt_profile.argtypes = [ctypes.POINTER(ctypes.c_int64),
                                           ctypes.c_size_t]
    lib.axon_start_nrt_profile.restype = ctypes.c_int64
    lib.axon_stop_nrt_profile.argtypes = [ctypes.c_char_p]
    lib.axon_stop_nrt_profile.restype = ctypes.c_int64

    def _mk():
        @contextlib.contextmanager
        def _hook(output_dir, device_ids):
            import jax
            jax.devices()
            if device_ids:
                ids = (ctypes.c_int64 * len(device_ids))(*device_ids)
                rc = lib.axon_start_nrt_profile(ids, len(device_ids))
            else:
                rc = lib.axon_start_nrt_profile(None, 0)
            if rc != 0:
                raise RuntimeError(f"axon_start_nrt_profile rc={rc}")
            try:
                yield
            finally:
                lib.axon_stop_nrt_profile(str(output_dir).encode())
        return _hook

    mod = types.ModuleType("antenv.axon_hooks")
    mod.get_axon_ntff_profile_hook = _mk
    sys.modules["antenv.axon_hooks"] = mod


def kernel(enc_in, dec_in, enc_W, enc_b, dec_W, dec_b, out_W, out_b):
    from concourse.bass_utils import run_bass_kernel_spmd

    trace = os.environ.get("KERNEL_TRACE", "") == "1"
    if trace:
        _install_trace_hook()

    use_bf16 = os.environ.get("KERNEL_DTYPE", "bf16") != "f32r"
    if "nc" not in _CACHE:
        _CACHE["nc"] = _build_program(use_bf16)
    nc = _CACHE["nc"]

    args = (np.asarray(enc_in, np.float32), np.asarray(dec_in, np.float32),
            np.asarray(enc_W, np.float32), np.asarray(enc_b, np.float32),
            np.asarray(dec_W, np.float32), np.asarray(dec_b, np.float32),
            np.asarray(out_W, np.float32), np.asarray(out_b, np.float32))
    in_maps = [_prep_core_inputs(c, *args, use_bf16=use_bf16)
               for c in range(NCORES)]

    res = run_bass_kernel_spmd(nc, in_maps, list(range(NCORES)), trace=trace)
    if trace:
        _CACHE["exec_time_ns"] = res.exec_time_ns

    B = enc_in.shape[0]
    out = np.empty((B, T, F, HS, WS), np.float32)
    for c in range(NCORES):
        b, half = c // 2, c % 2
        yc = res.results[c]["y"]  # [T, F, 32, 64]
        if half:
            out[b, :, :, 32:64, :] = yc[:, :, ::-1, :]
        else:
            out[b, :, :, 0:32, :] = yc
    return out


# revision 33
# speedup vs baseline: 1.1805x; 1.0218x over previous
"""EncDec ConvLSTM kernel for 8 Trainium2 NeuronCores.

Sharding: 8 cores = 4 (batch) x 2 (spatial row-halves). Each core owns 32
output rows; a 3-row halo is refreshed by a pairwise AllReduce exchange
every 3rd step, so per-step redundant compute shrinks from the old
no-comms scheme's 880 rows/core to 661. Row-half 1 cores receive a
vertically flipped image and ky-flipped conv weights, so a single SPMD
program serves all cores. Ghost rows are recovered rank-agnostically as
(pair-sum - mine); the exchange is issued right after the boundary tile
(computed first on exchange steps) so its ~13us latency hides under ~2.5
steps of compute.

Conv3x3 maps to PE matmuls over pixels (N up to 512, bf16), all with
full K=128 stationary loads so LDWEIGHTS hides behind in-flight matmuls.
Two double-buffered state tiles per step:
  R  = [h (parts 0:64) | h col-shifted +2 (parts 64:128)]
  R2 = [h (parts 0:64) | h row-shifted +1 (parts 64:128)]
Per row-tile and M-tile, 6 matmuls: x-im2col (K=72 zero-padded to 128),
3 paired-kx taps on R at row offsets 0/1/2, middle-column ky=0/1 as one
K=128 MM on R2, and ky=2 zero-padded on R. The three shift copies per
tile are flat contiguous-span DMAs (pad columns are zero, so a uniform
address delta realizes the shift), which issue far cheaper than strided
row views. The per-step x im2col loads are 3 batched DMAs (one per ky)
using a custom overlapping-window access pattern.

Epilogue runs entirely in tanh form (sigmoid(z) = 0.5*tanh(z/2)+0.5 with
the /2 folded into ACT scale/bias and the +1/x0.5 fixups into
scalar_tensor_tensor ops; stored h and c carry a 2x factor compensated
by pre-halved h-tap weights), 3 ACT ops per tile. The (f+1)*c product
runs on gpsimd to balance DVE load. Each tile's tail (tanh(c), h write,
shifts) is emitted one tile later so the ACT FIFO never head-of-line
blocks on the DVE c-update chain; decoder out-convs are interleaved
between gate tiles and their outputs batched to one DMA per step. A
32-matmul warm-up raises the HAM clock gate before the real work.
"""

import os
import sys

import numpy as np

for _p in ("/opt/trn_rl_repo", "/root/.axon_site/_ro/trn_rl_repo"):
    if os.path.isdir(_p) and _p not in sys.path:
        sys.path.append(_p)

T = 10
F = 8
HD = 64
HS = 64
WS = 64
NCORES = 8
PW = 66   # padded grid width
# LEAD is odd so interior writes (offset LEAD + r*66 + 1) are 4-byte
# aligned in bf16 -- required for the DVE 2x_1P perf mode on the h-writes
LEAD = 67
RSZ = LEAD + 38 * PW + 3   # flat elems per partition in R (rows 0..37)
XROWS = 38                 # padded x rows staged in DRAM
X2SZ = 36 * PW             # x im2col buffer elems per partition
NSTEPS = 2 * T
EX_STEPS = (4, 8, 12, 16)  # exchange after these steps
# ghost depth shipped by each exchange (the last one ships one extra row
# so the final step computes row 32 locally for its out conv)
EX_DEPTH = {4: 4, 8: 4, 12: 4, 16: 5}

_CACHE = {}


def _rows(s):
    """Computed rows at recurrent step s (1-based): 31 + halo depth."""
    if s >= 17:
        return 31 + 5 - (s - 17)
    return 31 + 4 - ((s - 1) % 4)


def _build_program(use_bf16=True):
    import bass_rust
    from concourse import bacc, mybir, tile

    F32 = mybir.dt.float32
    MMDT = mybir.dt.bfloat16 if use_bf16 else mybir.dt.float32r
    ACT = mybir.ActivationFunctionType
    ALU = mybir.AluOpType

    nc = bacc.Bacc("TRN2", target_bir_lowering=False, debug=False,
                   num_devices=NCORES)

    def din(name, shape, dt=MMDT):
        return nc.dram_tensor(name, shape, dt, kind="ExternalInput").ap()

    xe_d = din("xe", [T, F, XROWS, PW])
    xd_d = din("xd", [T, F, XROWS, PW])
    w_x = {"e": din("w_ex", [128, 256]), "d": din("w_dx", [128, 256])}
    w_p = {ph: [din(f"w_{ph}p{k}", [128, 256]) for k in range(3)]
           for ph in ("e", "d")}
    w_ma = {ph: din(f"w_{ph}ma", [128, 256]) for ph in ("e", "d")}
    w_mb = {ph: din(f"w_{ph}mb", [128, 256]) for ph in ("e", "d")}
    w_op = [din(f"w_op{k}", [128, 8]) for k in range(3)]
    w_oma = din("w_oma", [128, 8])
    w_omb = din("w_omb", [128, 8])
    scl_d = din("scl", [128, 1], F32)  # og tanh scale: 0.5 (o) / 1.0 (g)
    b_m0 = {"e": din("b_e0", [128, 1], F32), "d": din("b_d0", [128, 1], F32)}
    b_m1 = {"e": din("b_e1", [128, 1], F32), "d": din("b_d1", [128, 1], F32)}
    b_o = din("b_o", [8, 1], F32)
    y_d = nc.dram_tensor("y", [T, F, 32, WS], F32, kind="ExternalOutput").ap()

    groups = [[2 * i, 2 * i + 1] for i in range(4)]

    with tile.TileContext(nc) as tc:
        with tc.tile_pool(name="wpool", bufs=1) as wp, \
             tc.tile_pool(name="state", bufs=1) as stp, \
             tc.tile_pool(name="x2p", bufs=2) as x2p, \
             tc.tile_pool(name="gps", bufs=6, space="PSUM") as gps, \
             tc.tile_pool(name="ops", bufs=2, space="PSUM") as ops, \
             tc.tile_pool(name="fip", bufs=3) as fip, \
             tc.tile_pool(name="ogp", bufs=3) as ogp, \
             tc.tile_pool(name="t0p", bufs=3) as t0p, \
             tc.tile_pool(name="t1p", bufs=3) as t1p, \
             tc.tile_pool(name="thp", bufs=3) as thp, \
             tc.tile_pool(name="sxp", bufs=2) as sxp, \
             tc.tile_pool(name="dram", bufs=2, space="DRAM") as dram, \
             tc.tile_pool(name="yyp", bufs=2) as yyp:

            # ---- load weights / biases into SBUF ----
            def wtile(src, shape, tag, dt=MMDT):
                t_ = wp.tile(shape, dt, tag=tag)
                nc.sync.dma_start(t_[:], src[:])
                return t_

            sw_x, sw_p, sw_ma, sw_mb, sb_m0, sb_m1 = {}, {}, {}, {}, {}, {}
            # step-1-critical loads first: sw_x/scl/biases (skip_h step 1
            # needs only these), so x2col(1) isn't stuck behind the full
            # weight set on the sync queue
            sw_x["e"] = wtile(w_x["e"], [128, 256], "wxe")
            sw_op = [wtile(w_op[k], [128, 8], f"wop{k}") for k in range(3)]
            sscl = wtile(scl_d, [128, 1], "scl", F32)
            sb_m0["e"] = wtile(b_m0["e"], [128, 1], "b0e", F32)
            sb_m1["e"] = wtile(b_m1["e"], [128, 1], "b1e", F32)
            sw_p["e"] = [wtile(w_p["e"][k], [128, 256], f"wpe{k}")
                         for k in range(3)]
            sw_ma["e"] = wtile(w_ma["e"], [128, 256], "wmae")
            sw_mb["e"] = wtile(w_mb["e"], [128, 256], "wmbe")

            # ---- persistent state ----
            rrA = stp.tile([128, RSZ], MMDT, tag="rrA")
            rrB = stp.tile([128, RSZ], MMDT, tag="rrB")
            r2A = stp.tile([128, RSZ], MMDT, tag="r2A")
            r2B = stp.tile([128, RSZ], MMDT, tag="r2B")
            # c in bf16: keeps every epilogue DVE op all-16-bit (2x mode)
            c_t = stp.tile([64, 36 * 64], MMDT, tag="c")

            # PE clock warm-up: sustained matmul activity raises the HAM
            # clock gate before the real work starts.
            for _ in range(32):
                wu = ops.tile([8, 512], F32, tag="pso")
                nc.tensor.matmul(wu[:, 0:256], sw_op[0][:],
                                 sw_x["e"][:, 0:256],
                                 start=True, stop=True)

            def gview(t_, p0, p1, flat_off, nr=8):
                v = t_[p0:p1, flat_off:flat_off + nr * PW]
                v = v.rearrange("p (r c) -> p r c", c=PW)
                return v[:, 0:nr, 0:64]

            x2bufs = [x2p.tile([128, X2SZ], MMDT, tag="x2", name=f"x2{i}")
                      for i in range(2)]
            nc.vector.memset(x2bufs[1][64:128], 0.0)  # step 1 buffer first
            nc.vector.memset(x2bufs[0][64:128], 0.0)

            def emit_x2col(s):
                """Load x im2col for step s: partition (ky*3+kx)*8+ic holds
                the flat padded image shifted by ky*66+kx (contiguous)."""
                ph = "e" if s <= T else "d"
                t_idx = (s - 1) if ph == "e" else (s - 1 - T)
                x_src = xe_d if ph == "e" else xd_d
                ln = (_rows(s) - 1) * PW + 64
                x2 = x2bufs[s % 2]
                flat = x_src[t_idx].rearrange("a r c -> a (r c)")
                for tap in range(9):
                    sh = (tap // 3) * PW + (tap % 3)
                    nc.sync.dma_start(x2[tap * 8:(tap + 1) * 8, 0:ln],
                                      flat[:, sh:sh + ln])
                return x2

            def gate_mms(ps, wp3, wma, wmb, ms, R, R2, r0, nr):
                for k in range(3):
                    nc.tensor.matmul(
                        ps, wp3[k][:, ms],
                        gview(R, 0, 128, LEAD + (r0 + k) * PW, nr),
                        start=False, stop=False)
                nc.tensor.matmul(ps, wma[:, ms],
                                 gview(R2, 0, 128, LEAD + r0 * PW + 1, nr),
                                 start=False, stop=False)
                nc.tensor.matmul(ps, wmb[:, ms],
                                 gview(R, 0, 128, LEAD + (r0 + 2) * PW + 1,
                                       nr),
                                 start=False, stop=True)

            def emit_outconv1(s, R, R2, n2, Y):
                """relu(out conv) for decoder step s, rows 8*n2..8*n2+7,
                written into the step's batched y tile Y."""
                t_o = s - 1 - T
                r0 = n2 * 8
                pso = ops.tile([8, 512], F32, tag="pso")
                for k in range(3):
                    nc.tensor.matmul(pso[:], sw_op[k][:],
                                     gview(R, 0, 128, LEAD + (r0 + k) * PW),
                                     start=(k == 0), stop=False)
                nc.tensor.matmul(pso[:], sw_oma[:, :],
                                 gview(R2, 0, 128, LEAD + r0 * PW + 1),
                                 start=False, stop=False)
                nc.tensor.matmul(pso[:], sw_omb[:, :],
                                 gview(R, 0, 128, LEAD + (r0 + 2) * PW + 1),
                                 start=False, stop=True)
                nc.scalar.activation(Y[:, n2 * 512:(n2 + 1) * 512], pso[:],
                                     ACT.Relu, bias=sb_o[:])
                if n2 == 3:
                    nc.gpsimd.dma_start(
                        y_d[t_o],
                        Y[:].rearrange("p (r c) -> p r c", c=64))

            def gate_x(s, ph, x2v, r0, nr):
                """The x-im2col matmuls: no dependency on the previous
                step's tails, so they issue first and fill PE stalls at
                step boundaries and exchange waits."""
                skip_h = s == 1
                ps0 = gps.tile([128, 512], F32, tag="ps")
                ps1 = gps.tile([128, 512], F32, tag="ps")
                W = nr * 64
                nc.tensor.matmul(ps0[:, 0:W], sw_x[ph][:, 0:128],
                                 x2v[0:128, r0:r0 + nr, 0:64],
                                 start=True, stop=skip_h)
                nc.tensor.matmul(ps1[:, 0:W], sw_x[ph][:, 128:256],
                                 x2v[0:128, r0:r0 + nr, 0:64],
                                 start=True, stop=skip_h)
                return ps0, ps1

            def gate_block(s, ph, R_r, R2_r, R_w, R2_w, pre, r0, nr):
                skip_h = s == 1
                ps0, ps1 = pre
                W = nr * 64
                if not skip_h:
                    gate_mms(ps0[:, 0:W], sw_p[ph], sw_ma[ph],
                             sw_mb[ph], slice(0, 128),
                             R_r, R2_r, r0, nr)
                    gate_mms(ps1[:, 0:W], sw_p[ph], sw_ma[ph],
                             sw_mb[ph], slice(128, 256),
                             R_r, R2_r, r0, nr)

                # epilogue: M0=[f;i] via sigmoid (so the c-chain runs as
                # pure tensor_tensor ops in the DVE 2x mode), M1=[o;g] via
                # tanh with the o fixup folded into the 2x-h convention
                fi = fip.tile([128, 512], MMDT, tag="fi")
                og = ogp.tile([128, 512], MMDT, tag="og")
                nc.scalar.activation(fi[:, 0:W], ps0[:, 0:W], ACT.Sigmoid,
                                     bias=sb_m0[ph][:])
                nc.scalar.activation(og[:, 0:W], ps1[:, 0:W], ACT.Tanh,
                                     bias=sb_m1[ph][:], scale=sscl[:])
                cs = c_t[:, r0 * 64:r0 * 64 + W]
                if skip_h:
                    nc.vector.tensor_tensor(
                        cs, fi[64:128, 0:W], og[64:128, 0:W], ALU.mult)
                else:
                    t0 = t0p.tile([64, 512], MMDT, tag="t0")
                    nc.vector.tensor_tensor(
                        t0[:, 0:W], fi[0:64, 0:W], cs, ALU.mult)
                    t1 = t1p.tile([64, 512], MMDT, tag="t1")
                    nc.vector.tensor_tensor(
                        t1[:, 0:W], fi[64:128, 0:W], og[64:128, 0:W],
                        ALU.mult)
                    nc.vector.tensor_tensor(
                        cs, t0[:, 0:W], t1[:, 0:W], ALU.add)
                return (R_w, R2_w, r0, nr, og)

            def gate_tail(st):
                R_w, R2_w, r0, nr, og = st
                W = nr * 64
                cs = c_t[:, r0 * 64:r0 * 64 + W]
                th = thp.tile([64, 512], MMDT, tag="th")
                nc.scalar.activation(th[:, 0:W], cs, ACT.Tanh)
                thv = th[:, 0:W].rearrange("p (r c) -> p r c", c=64)
                ogv = og[0:64, 0:W].rearrange("p (r c) -> p r c", c=64)
                nc.vector.scalar_tensor_tensor(
                    gview(R_w, 0, 64, LEAD + (r0 + 1) * PW + 1, nr),
                    ogv, 1.0, thv, ALU.add, ALU.mult)
                # shift copies as flat contiguous spans (pad cols are zero,
                # so a uniform address delta realizes the shift; the spill
                # into neighbouring pad columns is never read). Issue cost
                # is spread across three otherwise-idle queues.
                base = LEAD + (r0 + 1) * PW
                n = nr * PW
                src = R_w[0:64, base:base + n]
                nc.sync.dma_start(R_w[64:128, base - 2:base - 2 + n], src)
                nc.scalar.dma_start(R2_w[0:64, base:base + n], src)
                nc.gpsimd.dma_start(R2_w[64:128, base - PW:base - PW + n],
                                    src)

            CCW = 5 * PW + 4 * 64  # h-lower rows 27..31 + c rows 28..31

            def emit_exchange_send(R_w, d):
                """Send side of the pairwise halo exchange: AllReduce my h
                (lower half) rows 27..31 and c rows 28..31 with the
                partner's (fixed-size payload: collectives need contiguous
                full-tile APs). Recovery happens in emit_exchange_recv,
                emitted mid-next-step so the waiting ops never head-of-line
                block the vector queue while the collective is in flight."""
                bi = dram.tile([64, CCW], MMDT, tag="ccin")
                bo = dram.tile([64, CCW], MMDT, tag="ccout")
                nc.scalar.dma_start(
                    bi[:, 0:5 * PW],
                    R_w[0:64, LEAD + 28 * PW:LEAD + 33 * PW])
                nc.scalar.dma_start(
                    bi[:, 5 * PW:CCW], c_t[:, 28 * 64:32 * 64])
                nc.gpsimd.collective_compute(
                    "AllReduce", ALU.add, replica_groups=groups,
                    ins=[bi[:].opt()], outs=[bo[:].opt()])
                S = sxp.tile([64, CCW], MMDT, tag="S")
                nc.scalar.dma_start(S[:], bo[:])
                return S

            def emit_exchange_recv(S, R_w, R2_w, d):
                """Recover partner rows as (sum - mine): ghost h rows
                32..31+d, ghost c rows 32..30+d; derive the col-shifted
                upper half and R2 ghosts locally."""
                for j in range(d):
                    nc.vector.tensor_tensor(
                        R_w[0:64, LEAD + (33 + j) * PW:
                            LEAD + (34 + j) * PW],
                        S[:, (4 - j) * PW:(5 - j) * PW],
                        R_w[0:64, LEAD + (32 - j) * PW:
                            LEAD + (33 - j) * PW],
                        ALU.subtract)
                for j in range(d - 1):
                    nc.vector.tensor_tensor(
                        c_t[:, (32 + j) * 64:(33 + j) * 64],
                        S[:, 5 * PW + (3 - j) * 64:
                          5 * PW + (4 - j) * 64],
                        c_t[:, (31 - j) * 64:(32 - j) * 64],
                        ALU.subtract)
                src = R_w[0:64, LEAD + 33 * PW:LEAD + (33 + d) * PW]
                nc.scalar.dma_start(
                    R_w[64:128, LEAD + 33 * PW - 2:
                        LEAD + (33 + d) * PW - 2], src)
                nc.scalar.dma_start(
                    R2_w[0:64, LEAD + 33 * PW:LEAD + (33 + d) * PW], src)
                nc.scalar.dma_start(
                    R2_w[64:128, LEAD + 32 * PW:LEAD + (32 + d) * PW], src)

            # warm-up AllReduce: pays the NRT first-collective setup cost
            # (~20us) during the PE warm-up instead of at step 3
            wbi = dram.tile([64, 8], MMDT, tag="wcc")
            wbo = dram.tile([64, 8], MMDT, tag="wcc2")
            nc.scalar.dma_start(wbi[:], sw_op[0][0:64, 0:8])
            nc.gpsimd.collective_compute(
                "AllReduce", ALU.add, replica_groups=groups,
                ins=[wbi[:].opt()], outs=[wbo[:].opt()])

            pend = None           # (tile_id, tail_state)
            pend_ex = None        # in-flight exchange awaiting recovery
            cur_Y = None
            x2_cur = emit_x2col(1)
            x2_nxt = emit_x2col(2)
            nc.gpsimd.memset(rrB[:], 0.0)
            nc.gpsimd.memset(r2B[:], 0.0)
            nc.vector.memset(rrA[:], 0.0)
            nc.vector.memset(r2A[:], 0.0)
            # remaining weights, behind the early x2cols on the sync queue
            sw_oma = wtile(w_oma, [128, 8], "woma")
            sw_omb = wtile(w_omb, [128, 8], "womb")
            sb_o = wtile(b_o, [8, 1], "bo", F32)
            sw_x["d"] = wtile(w_x["d"], [128, 256], "wxd")
            sw_p["d"] = [wtile(w_p["d"][k], [128, 256], f"wpd{k}")
                         for k in range(3)]
            sw_ma["d"] = wtile(w_ma["d"], [128, 256], "wmad")
            sw_mb["d"] = wtile(w_mb["d"], [128, 256], "wmbd")
            sb_m0["d"] = wtile(b_m0["d"], [128, 1], "b0d", F32)
            sb_m1["d"] = wtile(b_m1["d"], [128, 1], "b1d", F32)
            for s in range(1, NSTEPS + 1):
                ph = "e" if s <= T else "d"
                rows = _rows(s)
                tiles = [(0, 8), (8, 8), (16, 8), (24, 8)]
                if rows > 32:
                    tiles.append((32, rows - 32))
                send = s in EX_STEPS
                order = [0, 3, 1, 2] if send else list(range(len(tiles)))
                if s % 2 == 0:
                    R_r, R_w, R2_r, R2_w = rrA, rrB, r2A, r2B
                else:
                    R_r, R_w, R2_r, R2_w = rrB, rrA, r2B, r2A

                x2v = x2_cur[:].rearrange("p (r c) -> p r c", c=PW)
                if s == 1:
                    x2_next = x2_nxt  # prefetched before the weight bulk
                elif s < NSTEPS:
                    x2_next = emit_x2col(s + 1)

                for i, n in enumerate(order):
                    r0, nr = tiles[n]
                    # x matmuls first, then the prev decoder step's out
                    # conv: both independent of this step's tail chain, so
                    # they fill PE stalls at boundaries and ghost waits
                    pre = gate_x(s, ph, x2v, r0, nr)
                    if s > T + 1 and i < 4:
                        if i == 0:
                            cur_Y = yyp.tile([8, 2048], F32, tag="Y")
                        emit_outconv1(s - 1, R_r, R2_r, i, cur_Y)
                    st = gate_block(s, ph, R_r, R2_r, R_w, R2_w, pre,
                                    r0, nr)
                    if pend is not None:
                        gate_tail(pend[1])
                        if send and pend[0] == 3:
                            dd = EX_DEPTH[s]
                            pend_ex = (emit_exchange_send(R_w, dd),
                                       R_w, R2_w, dd)
                    pend = (n, st)
                    # ghost recovery for an in-flight exchange: emitted
                    # after this step's second tile so the waiting ops sit
                    # behind real work on the vector/scalar queues
                    if i == 1 and not send and pend_ex is not None:
                        emit_exchange_recv(*pend_ex)
                        pend_ex = None
                    # final step's out conv rows 0:24 only need tails 0-3
                    if s == NSTEPS and i == 4:
                        fin_Y = yyp.tile([8, 2048], F32, tag="Y")
                        for _n2 in range(3):
                            emit_outconv1(NSTEPS, R_w, R2_w, _n2, fin_Y)

                if pend is not None:
                    gate_tail(pend[1])
                    if send and pend[0] == 3:
                        emit_exchange(R_w, R2_w)
                    pend = None

                if s < NSTEPS:
                    x2_cur = x2_next

            emit_outconv1(NSTEPS, rrB, r2B, 3, fin_Y)

    nc.compile()
    return nc


def _prep_core_inputs(core, enc_in, dec_in, enc_W, enc_b, dec_W, dec_b,
                      out_W, out_b, use_bf16=True):
    import ml_dtypes
    mm_np = ml_dtypes.bfloat16 if use_bf16 else np.float32
    b, half = core // 2, core % 2
    # gate permutation: [f, i, o, g]
    perm = np.concatenate([np.arange(0, 128), np.arange(192, 256),
                           np.arange(128, 192)])

    def prep_x(x):
        x = x[b]  # [T, F, 64, 64]
        if half:
            x = x[:, :, ::-1, :]
        xp = np.zeros((T, F, XROWS, PW), np.float32)
        xp[:, :, 1:38, 1:65] = x[:, :, 0:37, :]
        return np.ascontiguousarray(xp)

    def prep_gateW(W, bias):
        Wf = W[:, :, ::-1, :] if half else W
        Wp = np.ascontiguousarray(Wf[perm])  # [256, 72, 3, 3]
        bp = bias[perm].astype(np.float32)
        lx = np.zeros((128, 256), np.float32)
        lx[0:72] = Wp[:, :F].transpose(2, 3, 1, 0).reshape(72, 256)
        # h-tap weights halved: stored h carries a 2x factor
        lp = [0.5 * np.concatenate(
            [Wp[:, F:, k, 0].T, Wp[:, F:, k, 2].T], axis=0)
            for k in range(3)]
        lma = 0.5 * np.concatenate([Wp[:, F:, 0, 1].T, Wp[:, F:, 1, 1].T],
                                   axis=0)
        lmb = np.zeros((128, 256), np.float32)
        lmb[0:64] = 0.5 * Wp[:, F:, 2, 1].T
        b0 = bp[0:128].copy()  # f,i biases plain: ACT0 is a true sigmoid
        b1 = np.concatenate([0.5 * bp[128:192], bp[192:256]])
        return (np.ascontiguousarray(lx),
                [np.ascontiguousarray(a) for a in lp],
                np.ascontiguousarray(lma), lmb,
                np.ascontiguousarray(b0.reshape(128, 1)),
                np.ascontiguousarray(b1.reshape(128, 1)))

    ex, ep, ema, emb, eb0, eb1 = prep_gateW(enc_W, enc_b)
    dx, dp, dma_, dmb, db0, db1 = prep_gateW(dec_W, dec_b)
    oWf = out_W[:, :, ::-1, :] if half else out_W
    op = [np.ascontiguousarray(0.5 * np.concatenate(
        [oWf[:, :, k, 0].T, oWf[:, :, k, 2].T], axis=0).astype(np.float32))
        for k in range(3)]
    oma = np.ascontiguousarray(0.5 * np.concatenate(
        [oWf[:, :, 0, 1].T, oWf[:, :, 1, 1].T], axis=0))
    omb = np.zeros((128, 8), np.float32)
    omb[0:64] = 0.5 * oWf[:, :, 2, 1].T
    scl = np.concatenate([np.full(64, 0.5, np.float32),
                          np.full(64, 1.0, np.float32)]).reshape(128, 1)

    m = {"xe": prep_x(enc_in), "xd": prep_x(dec_in),
         "w_ex": ex, "w_dx": dx,
         "w_ema": ema, "w_emb": emb, "w_dma": dma_, "w_dmb": dmb,
         "w_oma": oma, "w_omb": omb, "scl": scl,
         "b_e0": eb0, "b_e1": eb1, "b_d0": db0, "b_d1": db1,
         "b_o": np.ascontiguousarray(out_b.reshape(8, 1).astype(np.float32))}
    for k in range(3):
        m[f"w_ep{k}"] = ep[k]
        m[f"w_dp{k}"] = dp[k]
        m[f"w_op{k}"] = op[k]
    f32_keys = {"b_e0", "b_e1", "b_d0", "b_d1", "b_o", "scl"}
    return {k: np.ascontiguousarray(np.asarray(
        v, np.float32 if k in f32_keys else mm_np)) for k, v in m.items()}


def _install_trace_hook():
    """Shim antenv.axon_hooks for NTFF profiling (dev only)."""
    import contextlib
    import ctypes
    import types

    so = "/opt/axon/libaxon_pjrt.so"
    if "antenv.axon_hooks" in sys.modules or not os.path.exists(so):
        return
    lib = ctypes.CDLL(so)
    if not hasattr(lib, "axon_start_nrt_profile"):
        return
    lib.axon_start_nrt_profile.argtypes = [ctypes.POINTER(ctypes.c_int64),
                                           ctypes.c_size_t]
    lib.axon_start_nrt_profile.restype = ctypes.c_int64
    lib.axon_stop_nrt_profile.argtypes = [ctypes.c_char_p]
    lib.axon_stop_nrt_profile.restype = ctypes.c_int64

    def _mk():
        @contextlib.contextmanager
        def _hook(output_dir, device_ids):
            import jax
            jax.devices()
            if device_ids:
                ids = (ctypes.c_int64 * len(device_ids))(*device_ids)
                rc = lib.axon_start_nrt_profile(ids, len(device_ids))
            else:
                rc = lib.axon_start_nrt_profile(None, 0)
            if rc != 0:
                raise RuntimeError(f"axon_start_nrt_profile rc={rc}")
            try:
                yield
            finally:
                lib.axon_stop_nrt_profile(str(output_dir).encode())
        return _hook

    mod = types.ModuleType("antenv.axon_hooks")
    mod.get_axon_ntff_profile_hook = _mk
    sys.modules["antenv.axon_hooks"] = mod


def kernel(enc_in, dec_in, enc_W, enc_b, dec_W, dec_b, out_W, out_b):
    from concourse.bass_utils import run_bass_kernel_spmd

    trace = os.environ.get("KERNEL_TRACE", "") == "1"
    if trace:
        _install_trace_hook()

    use_bf16 = os.environ.get("KERNEL_DTYPE", "bf16") != "f32r"
    if "nc" not in _CACHE:
        _CACHE["nc"] = _build_program(use_bf16)
    nc = _CACHE["nc"]

    args = (np.asarray(enc_in, np.float32), np.asarray(dec_in, np.float32),
            np.asarray(enc_W, np.float32), np.asarray(enc_b, np.float32),
            np.asarray(dec_W, np.float32), np.asarray(dec_b, np.float32),
            np.asarray(out_W, np.float32), np.asarray(out_b, np.float32))
    in_maps = [_prep_core_inputs(c, *args, use_bf16=use_bf16)
               for c in range(NCORES)]

    res = run_bass_kernel_spmd(nc, in_maps, list(range(NCORES)), trace=trace)
    if trace:
        _CACHE["exec_time_ns"] = res.exec_time_ns

    B = enc_in.shape[0]
    out = np.empty((B, T, F, HS, WS), np.float32)
    for c in range(NCORES):
        b, half = c // 2, c % 2
        yc = res.results[c]["y"]  # [T, F, 32, 64]
        if half:
            out[b, :, :, 32:64, :] = yc[:, :, ::-1, :]
        else:
            out[b, :, :, 0:32, :] = yc
    return out
